# revision 4
# baseline (speedup 1.0000x reference)
"""Multi-head causal attention (B=2, S=2048, D=1024, H=16) on 8 TRN2 cores.

Sharding: core c handles batch c//4 and heads 4*(c%4) .. 4*(c%4)+4
(data-parallel over batch x head-group). Inside a core:
  - qT/kT = W{q,k}_slice @ X.T   (pair-packed, [128=2*dk, S] tiles)
  - v     = X @ Wv_slice.T       ([S, 4*dk], key-chunk major)
  - per head: causal scores -> exp(scale*s) on ScalarE with fused row-sum
    accumulate -> normalize -> DMA the valid (lower-triangular) blocks of
    the attention matrix; scores are recomputed transposed for the
    attn @ V matmul (keys on partitions), normalized via a PE-broadcast
    of the row reciprocals, and fed into the W_o projection.
Host gathers the 8 attention shards and sums the 4 partial output
projections per batch.
"""

import math

import numpy as np
import ml_dtypes

import concourse.bass as bass
from concourse import bacc
import concourse.mybir as mybir
import concourse.tile as tile
from concourse.bass_utils import run_bass_kernel_spmd

F32 = mybir.dt.float32
BF16 = mybir.dt.bfloat16
Exp = mybir.ActivationFunctionType.Exp
Mult = mybir.AluOpType.mult

B, S, D, H, DK = 2, 2048, 1024, 16, 64
HPC = 4  # heads per core
N_CORES = 8
SCALE = 1.0 / math.sqrt(DK)
TC = S // 128  # 16 token chunks
NEG = -1.0e9

# packed offsets for the transposed-exp buffer: block ci covers tokens
# ci*128 .. S with (TC-ci)*128 columns
ET_OFF = [0] * (TC + 1)
for _ci in range(TC):
    ET_OFF[_ci + 1] = ET_OFF[_ci] + (TC - _ci) * 128
ET_LEN = ET_OFF[TC]


def _build_nc():
    nc = bacc.Bacc(None)

    qt = nc.declare_dram_parameter("qt", [D, S], BF16, isOutput=False)
    kt = nc.declare_dram_parameter("kt", [D, S], BF16, isOutput=False)
    vt = nc.declare_dram_parameter("vt", [D, S], BF16, isOutput=False)
    wq = nc.declare_dram_parameter("wq", [D, HPC * DK], BF16, isOutput=False)
    wk = nc.declare_dram_parameter("wk", [D, HPC * DK], BF16, isOutput=False)
    wv = nc.declare_dram_parameter("wv", [D, HPC * DK], BF16, isOutput=False)
    wo = nc.declare_dram_parameter("wo", [HPC * DK, D], BF16, isOutput=False)
    maska = nc.declare_dram_parameter("maska", [128, 128], F32, isOutput=False)
    maskt = nc.declare_dram_parameter("maskt", [128, 128], F32, isOutput=False)
    ident = nc.declare_dram_parameter("ident", [128, 128], F32, isOutput=False)
    onesl = nc.declare_dram_parameter("onesl", [128, 64], F32, isOutput=False)

    attn_o = nc.declare_dram_parameter("attn", [HPC, S, S], F32, isOutput=True)
    out_o = nc.declare_dram_parameter("outp", [S, D], F32, isOutput=True)

    qt_r = qt.rearrange("(o p) f -> p o f", p=128)
    kt_r = kt.rearrange("(o p) f -> p o f", p=128)
    vt_r = vt.rearrange("(o p) f -> p o f", p=128)
    wq_r = wq.rearrange("(o p) f -> p o f", p=128)
    wk_r = wk.rearrange("(o p) f -> p o f", p=128)
    wv_r = wv.rearrange("(o p) f -> p o f", p=128)
    wo_r = wo.rearrange("(o p) f -> p o f", p=128)
    KC = D // 128  # 8 contraction chunks for the projections

    with tile.TileContext(nc) as tc:
        with (
            tc.tile_pool(name="consts", bufs=1) as consts,
            tc.tile_pool(name="qkpool", bufs=1) as qkpool,
            tc.tile_pool(name="esb", bufs=3) as esbp,
            tc.tile_pool(name="small", bufs=6) as small,
            tc.tile_pool(name="rbcp", bufs=2) as rbcp,
            tc.tile_pool(name="aop", bufs=1) as aop,
            tc.tile_pool(name="osbp", bufs=2) as osbp,
            tc.tile_pool(name="bigps", bufs=3, space="PSUM") as bigps,
            tc.tile_pool(name="smallps", bufs=1, space="PSUM") as smallps,
            tc.tile_pool(name="ptps", bufs=1, space="PSUM") as ptps,
        ):
            # ---- constants ----
            wq_sb = consts.tile([128, KC, HPC * DK], BF16, tag="wq")
            wk_sb = consts.tile([128, KC, HPC * DK], BF16, tag="wk")
            wv_sb = consts.tile([128, KC, HPC * DK], BF16, tag="wv")
            wo_sb = consts.tile([128, 2, D], BF16, tag="wo")
            maska_sb = consts.tile([128, 128], F32, tag="ma")
            maskt_sb = consts.tile([128, 128], F32, tag="mt")
            ident_sb = consts.tile([128, 128], F32, tag="id")
            ones_sb = consts.tile([128, 64], F32, tag="on")
            nc.sync.dma_start(out=wq_sb[:], in_=wq_r[:])
            nc.sync.dma_start(out=wk_sb[:], in_=wk_r[:])
            nc.sync.dma_start(out=wv_sb[:], in_=wv_r[:])
            nc.sync.dma_start(out=wo_sb[:], in_=wo_r[:])
            nc.sync.dma_start(out=maska_sb[:], in_=maska[:])
            nc.sync.dma_start(out=maskt_sb[:], in_=maskt[:])
            nc.sync.dma_start(out=ident_sb[:], in_=ident[:])
            nc.sync.dma_start(out=ones_sb[:], in_=onesl[:])

            # persistent per-core activation tensors
            q_pair = [qkpool.tile([128, S], BF16, tag=f"qp{p}", name=f"qp{p}") for p in range(2)]
            k_pair = [qkpool.tile([128, S], BF16, tag=f"kp{p}", name=f"kp{p}") for p in range(2)]
            v_all = qkpool.tile([128, TC, HPC * DK], BF16, tag="va")
            r_head = [qkpool.tile([128, TC], F32, tag=f"r{h}", name=f"r{h}") for h in range(HPC)]
            ao_pair = [aop.tile([128, S], BF16, tag=f"ao{p}", name=f"ao{p}") for p in range(2)]

            # ---- projections ----
            with tc.tile_pool(name="proj_in", bufs=2) as proj_in:
                def load_xt(src_r):
                    t_sb = proj_in.tile([128, KC, S], BF16, tag="pin")
                    for o in range(KC):
                        nc.sync.dma_start(out=t_sb[:, o, :], in_=src_r[:, o, :])
                    return t_sb

                def proj_qk(x_sb, w_sb, dst_pair):
                    for p in range(2):
                        for cc in range(2):
                            ps = bigps.tile([128, 1024], F32, tag="sps")
                            for k in range(KC):
                                lhs = w_sb[:, k, p * 128:(p + 1) * 128]
                                for nh in range(2):
                                    nc.tensor.matmul(
                                        ps[:, nh * 512:(nh + 1) * 512],
                                        lhs,
                                        x_sb[:, k, cc * 1024 + nh * 512:
                                             cc * 1024 + (nh + 1) * 512],
                                        start=(k == 0), stop=(k == KC - 1),
                                    )
                            nc.vector.tensor_copy(
                                dst_pair[p][:, cc * 1024:(cc + 1) * 1024], ps[:])

                x_sb = load_xt(qt_r)
                proj_qk(x_sb, wq_sb, q_pair)
                x_sb = load_xt(kt_r)
                proj_qk(x_sb, wk_sb, k_pair)
                x_sb = load_xt(vt_r)
                for kc in range(TC):
                    ps = smallps.tile([128, 512], F32, tag="mps")
                    for k in range(KC):
                        nc.tensor.matmul(
                            ps[:, 0:HPC * DK],
                            x_sb[:, k, kc * 128:(kc + 1) * 128],
                            wv_sb[:, k, :],
                            start=(k == 0), stop=(k == KC - 1),
                        )
                    nc.vector.tensor_copy(v_all[:, kc, :], ps[:, 0:HPC * DK])

            # ---- attention ----
            with tc.tile_pool(name="etp", bufs=2) as etp:
                for p in range(2):
                    qs = [q_pair[p][hh * 64:(hh + 1) * 64, :] for hh in range(2)]
                    ks = [k_pair[p][hh * 64:(hh + 1) * 64, :] for hh in range(2)]

                    # normal layout: scores -> softmax -> attn output.
                    # The two heads' K=64 matmuls are emitted back-to-back at
                    # PE row groups 0/64 so they pack into disjoint subarrays.
                    for t in range(TC):
                        L = (t + 1) * 128
                        nseg = (L + 1023) // 1024
                        e_sb = [esbp.tile([128, S], F32, tag="e", name=f"e{hh}")
                                for hh in range(2)]
                        acc = [small.tile([128, 2], F32, tag="acc",
                                          name=f"acc{hh}") for hh in range(2)]
                        dseg = (t * 128) // 1024
                        for s in range(nseg):
                            slen = min(1024, L - s * 1024)
                            pss = [bigps.tile([128, 1024], F32, tag="sps",
                                              name=f"sps{hh}") for hh in range(2)]
                            for n0 in range(0, slen, 512):
                                nn = min(512, slen - n0)
                                for hh in range(2):
                                    nc.tensor.matmul(
                                        pss[hh][:, n0:n0 + nn],
                                        qs[hh][:, t * 128:(t + 1) * 128],
                                        ks[hh][:, s * 1024 + n0:
                                                s * 1024 + n0 + nn],
                                        start=True, stop=True,
                                    )
                            for hh in range(2):
                                if s == dseg:
                                    lo = t * 128 - dseg * 1024
                                    nc.vector.tensor_add(
                                        pss[hh][:, lo:lo + 128],
                                        pss[hh][:, lo:lo + 128], maska_sb[:])
                                nc.scalar.activation(
                                    out=e_sb[hh][:, s * 1024:s * 1024 + slen],
                                    in_=pss[hh][:, 0:slen], func=Exp,
                                    scale=SCALE, accum_out=acc[hh][:, s:s + 1])
                        for hh in range(2):
                            h = 2 * p + hh
                            if nseg == 1:
                                ssum = acc[hh][:, 0:1]
                            else:
                                ssum = small.tile([128, 1], F32, tag="st")
                                nc.vector.reduce_sum(
                                    ssum[:], acc[hh][:, 0:2],
                                    axis=mybir.AxisListType.X)
                            nc.vector.reciprocal(r_head[h][:, t:t + 1], ssum[:])
                            nc.gpsimd.tensor_scalar_mul(
                                e_sb[hh][:, 0:L], e_sb[hh][:, 0:L],
                                r_head[h][:, t:t + 1])
                            nc.sync.dma_start(
                                out=attn_o[h, t * 128:(t + 1) * 128, 0:L],
                                in_=e_sb[hh][:, 0:L])

                    # transposed layout: exp(scores.T) for the AV matmul
                    et_h = [etp.tile([128, ET_LEN], BF16, tag="et",
                                     name=f"et{hh}") for hh in range(2)]
                    for ci in range(TC):
                        Lc = (TC - ci) * 128
                        nseg = (Lc + 1023) // 1024
                        for s in range(nseg):
                            slen = min(1024, Lc - s * 1024)
                            pss = [bigps.tile([128, 1024], F32, tag="sps",
                                              name=f"tsps{hh}")
                                   for hh in range(2)]
                            for n0 in range(0, slen, 512):
                                nn = min(512, slen - n0)
                                for hh in range(2):
                                    nc.tensor.matmul(
                                        pss[hh][:, n0:n0 + nn],
                                        ks[hh][:, ci * 128:(ci + 1) * 128],
                                        qs[hh][:, ci * 128 + s * 1024 + n0:
                                                ci * 128 + s * 1024 + n0 + nn],
                                        start=True, stop=True,
                                    )
                            for hh in range(2):
                                if s == 0:
                                    nc.vector.tensor_add(
                                        pss[hh][:, 0:128], pss[hh][:, 0:128],
                                        maskt_sb[:])
                                nc.scalar.activation(
                                    out=et_h[hh][:, ET_OFF[ci] + s * 1024:
                                                  ET_OFF[ci] + s * 1024 + slen],
                                    in_=pss[hh][:, 0:slen], func=Exp,
                                    scale=SCALE)

                    # r broadcast [128, S]: rows 0:64 <- r of head 2p per
                    # token, rows 64:128 <- r of head 2p+1
                    rbc_sb = rbcp.tile([128, S], F32, tag="rbc")
                    for tg in range(4):
                        rps = smallps.tile([128, 512], F32, tag="mps")
                        for tt in range(4):
                            t = tg * 4 + tt
                            for hh in range(2):
                                tmp = small.tile([128, 128], F32, tag="tmp")
                                nc.gpsimd.tensor_scalar_mul(
                                    tmp[:], ident_sb[:],
                                    r_head[2 * p + hh][:, t:t + 1])
                                nc.tensor.matmul(
                                    rps[hh * 64:(hh + 1) * 64,
                                        tt * 128:(tt + 1) * 128],
                                    ones_sb[:, 0:64], tmp[:],
                                    start=True, stop=True)
                        nc.vector.tensor_copy(
                            rbc_sb[:, tg * 512:(tg + 1) * 512], rps[:])

                    # attn @ V (transposed, unnormalized) + normalize
                    for n in range(4):
                        pt = ptps.tile([128, 512], F32, tag="pt")
                        last_ci = 4 * n + 3
                        for ci in range(last_ci + 1):
                            col0 = max(0, ci * 128 - n * 512)
                            g0 = n * 512 + col0
                            ln = 512 - col0
                            for hh in range(2):
                                nc.tensor.matmul(
                                    pt[hh * 64:(hh + 1) * 64, col0:512],
                                    v_all[:, ci, (2 * p + hh) * DK:
                                          (2 * p + hh + 1) * DK],
                                    et_h[hh][:, ET_OFF[ci] + g0 - ci * 128:
                                             ET_OFF[ci] + g0 - ci * 128 + ln],
                                    start=(ci == 0), stop=(ci == last_ci),
                                    skip_group_check=True,
                                )
                        nc.vector.tensor_tensor(
                            ao_pair[p][:, n * 512:(n + 1) * 512],
                            pt[:], rbc_sb[:, n * 512:(n + 1) * 512], Mult)

            # ---- output projection (partial, host sums across cores) ----
            for t in range(TC):
                osb = osbp.tile([128, D], F32, tag="osb")
                for nh in range(2):
                    wps = smallps.tile([128, 512], F32, tag="mps")
                    for p in range(2):
                        nc.tensor.matmul(
                            wps[:],
                            ao_pair[p][:, t * 128:(t + 1) * 128],
                            wo_sb[:, p, nh * 512:(nh + 1) * 512],
                            start=(p == 0), stop=(p == 1),
                        )
                    nc.vector.tensor_copy(osb[:, nh * 512:(nh + 1) * 512], wps[:])
                nc.sync.dma_start(
                    out=out_o[t * 128:(t + 1) * 128, :], in_=osb[:])

    nc.finalize()
    return nc


_NC = None


def _get_nc():
    global _NC
    if _NC is None:
        _NC = _build_nc()
    return _NC


def kernel(Q, K, V, W_q, W_k, W_v, W_o, b_o):
    Q = np.asarray(Q, dtype=np.float32)
    K = np.asarray(K, dtype=np.float32)
    V = np.asarray(V, dtype=np.float32)
    W_q = np.asarray(W_q, dtype=np.float32)
    W_k = np.asarray(W_k, dtype=np.float32)
    W_v = np.asarray(W_v, dtype=np.float32)
    W_o = np.asarray(W_o, dtype=np.float32)
    b_o = np.asarray(b_o, dtype=np.float32)

    bf = ml_dtypes.bfloat16
    ii = np.arange(128)
    maska = np.where(ii[None, :] <= ii[:, None], 0.0, NEG).astype(np.float32)
    maskt = maska.T.copy()
    ident = np.eye(128, dtype=np.float32)
    onesl = np.ones((128, 64), dtype=np.float32)

    xts = [[x[b].T.astype(bf) for x in (Q, K, V)] for b in range(B)]
    in_maps = []
    for c in range(N_CORES):
        b = c // 4
        g = c % 4
        osl = slice(g * HPC * DK, (g + 1) * HPC * DK)
        in_maps.append({
            "qt": xts[b][0], "kt": xts[b][1], "vt": xts[b][2],
            "wq": W_q[osl, :].T.astype(bf),
            "wk": W_k[osl, :].T.astype(bf),
            "wv": W_v[osl, :].T.astype(bf),
            "wo": W_o[:, osl].T.astype(bf),
            "maska": maska, "maskt": maskt, "ident": ident, "onesl": onesl,
        })

    nc = _get_nc()
    res = run_bass_kernel_spmd(nc, in_maps, core_ids=list(range(N_CORES)))

    attn = np.zeros((B, H, S, S), dtype=np.float32)
    out = np.zeros((B, S, D), dtype=np.float32)
    for c in range(N_CORES):
        b = c // 4
        g = c % 4
        attn[b, g * HPC:(g + 1) * HPC] = res.results[c]["attn"]
        out[b] += res.results[c]["outp"]
    # enforce exact zeros above the causal diagonal (the device never
    # writes there; this guards against non-zeroed output buffers)
    for t in range(TC):
        attn[:, :, t * 128:(t + 1) * 128, (t + 1) * 128:] = 0.0
    out += b_o
    return out, attn


# revision 5
# speedup vs baseline: 3.8495x; 3.8495x over previous
"""Multi-head causal attention (B=2, S=2048, D=1024, H=16) on 8 TRN2 cores.

Sharding: core c handles batch c//4 and heads 4*(c%4) .. 4*(c%4)+4
(data-parallel over batch x head-group). Inside a core:
  - qT/kT = W{q,k}_slice @ X.T   (pair-packed, [128=2*dk, S] tiles)
  - v     = X @ Wv_slice.T       ([S, 4*dk], key-chunk major)
  - per head: causal scores -> exp(scale*s) on ScalarE with fused row-sum
    accumulate -> normalize -> DMA the valid (lower-triangular) blocks of
    the attention matrix; scores are recomputed transposed for the
    attn @ V matmul (keys on partitions), normalized via a PE-broadcast
    of the row reciprocals, and fed into the W_o projection.
Host gathers the 8 attention shards and sums the 4 partial output
projections per batch.
"""

import math

import numpy as np
import ml_dtypes

import concourse.bass as bass
from concourse import bacc
import concourse.mybir as mybir
import concourse.tile as tile
from concourse.bass_utils import run_bass_kernel_spmd

F32 = mybir.dt.float32
BF16 = mybir.dt.bfloat16
Exp = mybir.ActivationFunctionType.Exp
Mult = mybir.AluOpType.mult

B, S, D, H, DK = 2, 2048, 1024, 16, 64
HPC = 4  # heads per core
N_CORES = 8
SCALE = 1.0 / math.sqrt(DK)
TC = S // 128  # 16 token chunks
NEG = -1.0e9

# packed offsets for the transposed-exp buffer: block ci covers tokens
# ci*128 .. S with (TC-ci)*128 columns
ET_OFF = [0] * (TC + 1)
for _ci in range(TC):
    ET_OFF[_ci + 1] = ET_OFF[_ci] + (TC - _ci) * 128
ET_LEN = ET_OFF[TC]


def _build_nc():
    nc = bacc.Bacc(None)

    qt = nc.declare_dram_parameter("qt", [D, S], BF16, isOutput=False)
    kt = nc.declare_dram_parameter("kt", [D, S], BF16, isOutput=False)
    vt = nc.declare_dram_parameter("vt", [D, S], BF16, isOutput=False)
    wq = nc.declare_dram_parameter("wq", [D, HPC * DK], BF16, isOutput=False)
    wk = nc.declare_dram_parameter("wk", [D, HPC * DK], BF16, isOutput=False)
    wv = nc.declare_dram_parameter("wv", [D, HPC * DK], BF16, isOutput=False)
    wo = nc.declare_dram_parameter("wo", [HPC * DK, D], BF16, isOutput=False)
    maska = nc.declare_dram_parameter("maska", [128, 128], F32, isOutput=False)
    maskt = nc.declare_dram_parameter("maskt", [128, 128], F32, isOutput=False)
    ident = nc.declare_dram_parameter("ident", [128, 128], F32, isOutput=False)
    onesl = nc.declare_dram_parameter("onesl", [128, 64], F32, isOutput=False)

    attn_o = nc.declare_dram_parameter("attn", [HPC, S, S], F32, isOutput=True)
    out_o = nc.declare_dram_parameter("outp", [S, D], F32, isOutput=True)

    qt_r = qt.rearrange("(o p) f -> p o f", p=128)
    kt_r = kt.rearrange("(o p) f -> p o f", p=128)
    vt_r = vt.rearrange("(o p) f -> p o f", p=128)
    wq_r = wq.rearrange("(o p) f -> p o f", p=128)
    wk_r = wk.rearrange("(o p) f -> p o f", p=128)
    wv_r = wv.rearrange("(o p) f -> p o f", p=128)
    wo_r = wo.rearrange("(o p) f -> p o f", p=128)
    KC = D // 128  # 8 contraction chunks for the projections

    with tile.TileContext(nc) as tc:
        with (
            tc.tile_pool(name="consts", bufs=1) as consts,
            tc.tile_pool(name="qkpool", bufs=1) as qkpool,
            tc.tile_pool(name="esb", bufs=3) as esbp,
            tc.tile_pool(name="small", bufs=6) as small,
            tc.tile_pool(name="rbcp", bufs=2) as rbcp,
            tc.tile_pool(name="aop", bufs=1) as aop,
            tc.tile_pool(name="osbp", bufs=2) as osbp,
            tc.tile_pool(name="bigps", bufs=3, space="PSUM") as bigps,
            tc.tile_pool(name="smallps", bufs=1, space="PSUM") as smallps,
            tc.tile_pool(name="ptps", bufs=1, space="PSUM") as ptps,
        ):
            # ---- constants ----
            wq_sb = consts.tile([128, KC, HPC * DK], BF16, tag="wq")
            wk_sb = consts.tile([128, KC, HPC * DK], BF16, tag="wk")
            wv_sb = consts.tile([128, KC, HPC * DK], BF16, tag="wv")
            wo_sb = consts.tile([128, 2, D], BF16, tag="wo")
            maska_sb = consts.tile([128, 128], F32, tag="ma")
            maskt_sb = consts.tile([128, 128], F32, tag="mt")
            ident_sb = consts.tile([128, 128], F32, tag="id")
            ones_sb = consts.tile([128, 64], F32, tag="on")
            nc.sync.dma_start(out=wq_sb[:], in_=wq_r[:])
            nc.sync.dma_start(out=wk_sb[:], in_=wk_r[:])
            nc.sync.dma_start(out=wv_sb[:], in_=wv_r[:])
            nc.sync.dma_start(out=wo_sb[:], in_=wo_r[:])
            nc.sync.dma_start(out=maska_sb[:], in_=maska[:])
            nc.sync.dma_start(out=maskt_sb[:], in_=maskt[:])
            nc.sync.dma_start(out=ident_sb[:], in_=ident[:])
            nc.sync.dma_start(out=ones_sb[:], in_=onesl[:])

            # persistent per-core activation tensors
            q_pair = [qkpool.tile([128, S], BF16, tag=f"qp{p}", name=f"qp{p}") for p in range(2)]
            k_pair = [qkpool.tile([128, S], BF16, tag=f"kp{p}", name=f"kp{p}") for p in range(2)]
            v_all = qkpool.tile([128, TC, HPC * DK], BF16, tag="va")
            r_head = [qkpool.tile([128, TC], F32, tag=f"r{h}", name=f"r{h}") for h in range(HPC)]
            ao_pair = [aop.tile([128, S], BF16, tag=f"ao{p}", name=f"ao{p}") for p in range(2)]

            # ---- projections ----
            with tc.tile_pool(name="proj_in", bufs=2) as proj_in:
                def load_xt(src_r):
                    t_sb = proj_in.tile([128, KC, S], BF16, tag="pin")
                    for o in range(KC):
                        nc.sync.dma_start(out=t_sb[:, o, :], in_=src_r[:, o, :])
                    return t_sb

                def proj_qk(x_sb, w_sb, dst_pair):
                    for p in range(2):
                        for cc in range(2):
                            ps = bigps.tile([128, 1024], F32, tag="sps")
                            for k in range(KC):
                                lhs = w_sb[:, k, p * 128:(p + 1) * 128]
                                for nh in range(2):
                                    nc.tensor.matmul(
                                        ps[:, nh * 512:(nh + 1) * 512],
                                        lhs,
                                        x_sb[:, k, cc * 1024 + nh * 512:
                                             cc * 1024 + (nh + 1) * 512],
                                        start=(k == 0), stop=(k == KC - 1),
                                    )
                            nc.vector.tensor_copy(
                                dst_pair[p][:, cc * 1024:(cc + 1) * 1024], ps[:])

                x_sb = load_xt(qt_r)
                proj_qk(x_sb, wq_sb, q_pair)
                x_sb = load_xt(kt_r)
                proj_qk(x_sb, wk_sb, k_pair)
                x_sb = load_xt(vt_r)
                for kc in range(TC):
                    ps = smallps.tile([128, 512], F32, tag="mps")
                    for k in range(KC):
                        nc.tensor.matmul(
                            ps[:, 0:HPC * DK],
                            x_sb[:, k, kc * 128:(kc + 1) * 128],
                            wv_sb[:, k, :],
                            start=(k == 0), stop=(k == KC - 1),
                        )
                    nc.vector.tensor_copy(v_all[:, kc, :], ps[:, 0:HPC * DK])

            # ---- attention ----
            with tc.tile_pool(name="etp", bufs=2) as etp:
                for p in range(2):
                    qs = [q_pair[p][hh * 64:(hh + 1) * 64, :] for hh in range(2)]
                    ks = [k_pair[p][hh * 64:(hh + 1) * 64, :] for hh in range(2)]

                    # normal layout: scores -> softmax -> attn output.
                    # The two heads' K=64 matmuls are emitted back-to-back at
                    # PE row groups 0/64 so they pack into disjoint subarrays.
                    for t in range(TC):
                        L = (t + 1) * 128
                        nseg = (L + 1023) // 1024
                        e_sb = [esbp.tile([128, S], F32, tag="e", name=f"e{hh}")
                                for hh in range(2)]
                        acc = [small.tile([128, 2], F32, tag="acc",
                                          name=f"acc{hh}") for hh in range(2)]
                        dseg = (t * 128) // 1024
                        for s in range(nseg):
                            slen = min(1024, L - s * 1024)
                            pss = [bigps.tile([128, 1024], F32, tag="sps",
                                              name=f"sps{hh}") for hh in range(2)]
                            for n0 in range(0, slen, 512):
                                nn = min(512, slen - n0)
                                for hh in range(2):
                                    nc.tensor.matmul(
                                        pss[hh][:, n0:n0 + nn],
                                        qs[hh][:, t * 128:(t + 1) * 128],
                                        ks[hh][:, s * 1024 + n0:
                                                s * 1024 + n0 + nn],
                                        start=True, stop=True,
                                    )
                            for hh in range(2):
                                if s == dseg:
                                    lo = t * 128 - dseg * 1024
                                    nc.vector.tensor_add(
                                        pss[hh][:, lo:lo + 128],
                                        pss[hh][:, lo:lo + 128], maska_sb[:])
                                nc.scalar.activation(
                                    out=e_sb[hh][:, s * 1024:s * 1024 + slen],
                                    in_=pss[hh][:, 0:slen], func=Exp,
                                    scale=SCALE, accum_out=acc[hh][:, s:s + 1])
                        for hh in range(2):
                            h = 2 * p + hh
                            if nseg == 1:
                                ssum = acc[hh][:, 0:1]
                            else:
                                ssum = small.tile([128, 1], F32, tag="st")
                                nc.vector.reduce_sum(
                                    ssum[:], acc[hh][:, 0:2],
                                    axis=mybir.AxisListType.X)
                            nc.vector.reciprocal(r_head[h][:, t:t + 1], ssum[:])
                            nc.vector.tensor_scalar_mul(
                                e_sb[hh][:, 0:L], e_sb[hh][:, 0:L],
                                r_head[h][:, t:t + 1])
                            nc.sync.dma_start(
                                out=attn_o[h, t * 128:(t + 1) * 128, 0:L],
                                in_=e_sb[hh][:, 0:L])

                    # transposed layout: exp(scores.T) for the AV matmul
                    et_h = [etp.tile([128, ET_LEN], BF16, tag="et",
                                     name=f"et{hh}") for hh in range(2)]
                    for ci in range(TC):
                        Lc = (TC - ci) * 128
                        nseg = (Lc + 1023) // 1024
                        for s in range(nseg):
                            slen = min(1024, Lc - s * 1024)
                            pss = [bigps.tile([128, 1024], F32, tag="sps",
                                              name=f"tsps{hh}")
                                   for hh in range(2)]
                            for n0 in range(0, slen, 512):
                                nn = min(512, slen - n0)
                                for hh in range(2):
                                    nc.tensor.matmul(
                                        pss[hh][:, n0:n0 + nn],
                                        ks[hh][:, ci * 128:(ci + 1) * 128],
                                        qs[hh][:, ci * 128 + s * 1024 + n0:
                                                ci * 128 + s * 1024 + n0 + nn],
                                        start=True, stop=True,
                                    )
                            for hh in range(2):
                                if s == 0:
                                    nc.vector.tensor_add(
                                        pss[hh][:, 0:128], pss[hh][:, 0:128],
                                        maskt_sb[:])
                                nc.scalar.activation(
                                    out=et_h[hh][:, ET_OFF[ci] + s * 1024:
                                                  ET_OFF[ci] + s * 1024 + slen],
                                    in_=pss[hh][:, 0:slen], func=Exp,
                                    scale=SCALE)

                    # r broadcast [128, S]: rows 0:64 <- r of head 2p per
                    # token, rows 64:128 <- r of head 2p+1
                    rbc_sb = rbcp.tile([128, S], F32, tag="rbc")
                    for tg in range(4):
                        rps = smallps.tile([128, 512], F32, tag="mps")
                        for tt in range(4):
                            t = tg * 4 + tt
                            for hh in range(2):
                                tmp = small.tile([128, 128], F32, tag="tmp")
                                nc.vector.tensor_scalar_mul(
                                    tmp[:], ident_sb[:],
                                    r_head[2 * p + hh][:, t:t + 1])
                                nc.tensor.matmul(
                                    rps[hh * 64:(hh + 1) * 64,
                                        tt * 128:(tt + 1) * 128],
                                    ones_sb[:, 0:64], tmp[:],
                                    start=True, stop=True)
                        nc.vector.tensor_copy(
                            rbc_sb[:, tg * 512:(tg + 1) * 512], rps[:])

                    # attn @ V (transposed, unnormalized) + normalize
                    for n in range(4):
                        pt = ptps.tile([128, 512], F32, tag="pt")
                        last_ci = 4 * n + 3
                        for ci in range(last_ci + 1):
                            col0 = max(0, ci * 128 - n * 512)
                            g0 = n * 512 + col0
                            ln = 512 - col0
                            for hh in range(2):
                                nc.tensor.matmul(
                                    pt[hh * 64:(hh + 1) * 64, col0:512],
                                    v_all[:, ci, (2 * p + hh) * DK:
                                          (2 * p + hh + 1) * DK],
                                    et_h[hh][:, ET_OFF[ci] + g0 - ci * 128:
                                             ET_OFF[ci] + g0 - ci * 128 + ln],
                                    start=(ci == 0), stop=(ci == last_ci),
                                    skip_group_check=True,
                                )
                        nc.vector.tensor_tensor(
                            ao_pair[p][:, n * 512:(n + 1) * 512],
                            pt[:], rbc_sb[:, n * 512:(n + 1) * 512], Mult)

            # ---- output projection (partial, host sums across cores) ----
            for t in range(TC):
                osb = osbp.tile([128, D], F32, tag="osb")
                for nh in range(2):
                    wps = smallps.tile([128, 512], F32, tag="mps")
                    for p in range(2):
                        nc.tensor.matmul(
                            wps[:],
                            ao_pair[p][:, t * 128:(t + 1) * 128],
                            wo_sb[:, p, nh * 512:(nh + 1) * 512],
                            start=(p == 0), stop=(p == 1),
                        )
                    nc.vector.tensor_copy(osb[:, nh * 512:(nh + 1) * 512], wps[:])
                nc.sync.dma_start(
                    out=out_o[t * 128:(t + 1) * 128, :], in_=osb[:])

    nc.finalize()
    return nc


_NC = None


def _get_nc():
    global _NC
    if _NC is None:
        _NC = _build_nc()
    return _NC


def kernel(Q, K, V, W_q, W_k, W_v, W_o, b_o):
    Q = np.asarray(Q, dtype=np.float32)
    K = np.asarray(K, dtype=np.float32)
    V = np.asarray(V, dtype=np.float32)
    W_q = np.asarray(W_q, dtype=np.float32)
    W_k = np.asarray(W_k, dtype=np.float32)
    W_v = np.asarray(W_v, dtype=np.float32)
    W_o = np.asarray(W_o, dtype=np.float32)
    b_o = np.asarray(b_o, dtype=np.float32)

    bf = ml_dtypes.bfloat16
    ii = np.arange(128)
    maska = np.where(ii[None, :] <= ii[:, None], 0.0, NEG).astype(np.float32)
    maskt = maska.T.copy()
    ident = np.eye(128, dtype=np.float32)
    onesl = np.ones((128, 64), dtype=np.float32)

    xts = [[x[b].T.astype(bf) for x in (Q, K, V)] for b in range(B)]
    in_maps = []
    for c in range(N_CORES):
        b = c // 4
        g = c % 4
        osl = slice(g * HPC * DK, (g + 1) * HPC * DK)
        in_maps.append({
            "qt": xts[b][0], "kt": xts[b][1], "vt": xts[b][2],
            "wq": W_q[osl, :].T.astype(bf),
            "wk": W_k[osl, :].T.astype(bf),
            "wv": W_v[osl, :].T.astype(bf),
            "wo": W_o[:, osl].T.astype(bf),
            "maska": maska, "maskt": maskt, "ident": ident, "onesl": onesl,
        })

    nc = _get_nc()
    res = run_bass_kernel_spmd(nc, in_maps, core_ids=list(range(N_CORES)))

    attn = np.zeros((B, H, S, S), dtype=np.float32)
    out = np.zeros((B, S, D), dtype=np.float32)
    for c in range(N_CORES):
        b = c // 4
        g = c % 4
        attn[b, g * HPC:(g + 1) * HPC] = res.results[c]["attn"]
        out[b] += res.results[c]["outp"]
    # enforce exact zeros above the causal diagonal (the device never
    # writes there; this guards against non-zeroed output buffers)
    for t in range(TC):
        attn[:, :, t * 128:(t + 1) * 128, (t + 1) * 128:] = 0.0
    out += b_o
    return out, attn


# revision 7
# speedup vs baseline: 3.9234x; 1.0192x over previous
"""Multi-head causal attention (B=2, S=2048, D=1024, H=16) on 8 TRN2 cores.

Sharding: core c handles batch c//4 and heads 4*(c%4) .. 4*(c%4)+4
(data-parallel over batch x head-group). Inside a core:
  - qT/kT = W{q,k}_slice @ X.T   (pair-packed, [128=2*dk, S] tiles)
  - v     = X @ Wv_slice.T       ([S, 4*dk], key-chunk major)
  - per head: causal scores -> exp(scale*s) on ScalarE with fused row-sum
    accumulate -> normalize -> DMA the valid (lower-triangular) blocks of
    the attention matrix; scores are recomputed transposed for the
    attn @ V matmul (keys on partitions), normalized via a PE-broadcast
    of the row reciprocals, and fed into the W_o projection.
Host gathers the 8 attention shards and sums the 4 partial output
projections per batch.

Matmuls that share a stationary operand are emitted consecutively so
walrus folds their LDWEIGHTS and the PE streams them back-to-back
(ldw-opt is disabled in this toolchain, so every weight change costs a
serializing reload).
"""

import math

import numpy as np
import ml_dtypes

import concourse.bass as bass
from concourse import bacc
import concourse.mybir as mybir
import concourse.tile as tile
from concourse.bass_utils import run_bass_kernel_spmd

F32 = mybir.dt.float32
BF16 = mybir.dt.bfloat16
Exp = mybir.ActivationFunctionType.Exp
Mult = mybir.AluOpType.mult

B, S, D, H, DK = 2, 2048, 1024, 16, 64
HPC = 4  # heads per core
N_CORES = 8
SCALE = 1.0 / math.sqrt(DK)
TC = S // 128  # 16 token chunks
NEG = -1.0e9

# packed offsets for the transposed-exp buffer: block ci covers tokens
# ci*128 .. S with (TC-ci)*128 columns
ET_OFF = [0] * (TC + 1)
for _ci in range(TC):
    ET_OFF[_ci + 1] = ET_OFF[_ci] + (TC - _ci) * 128
ET_LEN = ET_OFF[TC]


def _build_nc():
    nc = bacc.Bacc(None)

    qt = nc.declare_dram_parameter("qt", [D, S], BF16, isOutput=False)
    kt = nc.declare_dram_parameter("kt", [D, S], BF16, isOutput=False)
    vt = nc.declare_dram_parameter("vt", [D, S], BF16, isOutput=False)
    wq = nc.declare_dram_parameter("wq", [D, HPC * DK], BF16, isOutput=False)
    wk = nc.declare_dram_parameter("wk", [D, HPC * DK], BF16, isOutput=False)
    wv = nc.declare_dram_parameter("wv", [D, HPC * DK], BF16, isOutput=False)
    wo = nc.declare_dram_parameter("wo", [HPC * DK, D], BF16, isOutput=False)
    maska = nc.declare_dram_parameter("maska", [128, 128], F32, isOutput=False)
    maskt = nc.declare_dram_parameter("maskt", [128, 128], F32, isOutput=False)
    ident = nc.declare_dram_parameter("ident", [128, 128], F32, isOutput=False)
    onesl = nc.declare_dram_parameter("onesl", [128, 64], F32, isOutput=False)

    attn_o = nc.declare_dram_parameter("attn", [HPC, S, S], F32, isOutput=True)
    out_o = nc.declare_dram_parameter("outp", [S, D], F32, isOutput=True)

    qt_r = qt.rearrange("(o p) f -> p o f", p=128)
    kt_r = kt.rearrange("(o p) f -> p o f", p=128)
    vt_r = vt.rearrange("(o p) f -> p o f", p=128)
    wq_r = wq.rearrange("(o p) f -> p o f", p=128)
    wk_r = wk.rearrange("(o p) f -> p o f", p=128)
    wv_r = wv.rearrange("(o p) f -> p o f", p=128)
    wo_r = wo.rearrange("(o p) f -> p o f", p=128)
    KC = D // 128  # 8 contraction chunks for the projections

    with tile.TileContext(nc) as tc:
        with (
            tc.tile_pool(name="consts", bufs=1) as consts,
            tc.tile_pool(name="qkpool", bufs=1) as qkpool,
            tc.tile_pool(name="esb", bufs=4) as esbp,
            tc.tile_pool(name="small", bufs=6) as small,
            tc.tile_pool(name="rbcp", bufs=2) as rbcp,
            tc.tile_pool(name="aop", bufs=1) as aop,
            tc.tile_pool(name="osbp", bufs=2) as osbp,
        ):
            # ---- constants ----
            wq_sb = consts.tile([128, KC, HPC * DK], BF16, tag="wq")
            wk_sb = consts.tile([128, KC, HPC * DK], BF16, tag="wk")
            wv_sb = consts.tile([128, KC, HPC * DK], BF16, tag="wv")
            wo_sb = consts.tile([128, 2, D], BF16, tag="wo")
            maska_sb = consts.tile([128, 128], F32, tag="ma")
            maskt_sb = consts.tile([128, 128], F32, tag="mt")
            ident_sb = consts.tile([128, 128], F32, tag="id")
            ones_sb = consts.tile([128, 64], F32, tag="on")
            nc.sync.dma_start(out=wq_sb[:], in_=wq_r[:])
            nc.sync.dma_start(out=wk_sb[:], in_=wk_r[:])
            nc.sync.dma_start(out=wv_sb[:], in_=wv_r[:])
            nc.sync.dma_start(out=wo_sb[:], in_=wo_r[:])
            nc.sync.dma_start(out=maska_sb[:], in_=maska[:])
            nc.sync.dma_start(out=maskt_sb[:], in_=maskt[:])
            nc.sync.dma_start(out=ident_sb[:], in_=ident[:])
            nc.sync.dma_start(out=ones_sb[:], in_=onesl[:])

            # persistent per-core activation tensors
            q_pair = [qkpool.tile([128, S], BF16, tag=f"qp{p}", name=f"qp{p}")
                      for p in range(2)]
            k_pair = [qkpool.tile([128, S], BF16, tag=f"kp{p}", name=f"kp{p}")
                      for p in range(2)]
            v_all = qkpool.tile([128, TC, HPC * DK], BF16, tag="va")
            r_head = [qkpool.tile([128, TC], F32, tag=f"r{h}", name=f"r{h}")
                      for h in range(HPC)]
            ao_pair = [aop.tile([128, S], BF16, tag=f"ao{p}", name=f"ao{p}")
                      for p in range(2)]

            # ---- projections ----
            with tc.tile_pool(name="proj_in", bufs=2) as proj_in:
                def load_xt(src_r):
                    t_sb = proj_in.tile([128, KC, S], BF16, tag="pin")
                    for o in range(KC):
                        nc.sync.dma_start(out=t_sb[:, o, :], in_=src_r[:, o, :])
                    return t_sb

                with tc.tile_pool(name="pps", bufs=2, space="PSUM") as pps:
                    def proj_qk(x_sb, w_sb, dst_pair):
                        # one [128, 2048] accumulator per head pair; each
                        # K-chunk issues 4 same-weights matmuls
                        for p in range(2):
                            ps = pps.tile([128, 2048], F32, tag="pp")
                            for k in range(KC):
                                lhs = w_sb[:, k, p * 128:(p + 1) * 128]
                                for nh in range(4):
                                    nc.tensor.matmul(
                                        ps[:, nh * 512:(nh + 1) * 512],
                                        lhs,
                                        x_sb[:, k, nh * 512:(nh + 1) * 512],
                                        start=(k == 0), stop=(k == KC - 1),
                                    )
                            nc.vector.tensor_copy(dst_pair[p][:], ps[:])

                    x_sb = load_xt(qt_r)
                    proj_qk(x_sb, wq_sb, q_pair)
                    x_sb = load_xt(kt_r)
                    proj_qk(x_sb, wk_sb, k_pair)
                x_sb = load_xt(vt_r)
                with tc.tile_pool(name="vps", bufs=2, space="PSUM") as vps:
                    for kc in range(TC):
                        ps = vps.tile([128, 512], F32, tag="vp")
                        for k in range(KC):
                            nc.tensor.matmul(
                                ps[:, 0:HPC * DK],
                                x_sb[:, k, kc * 128:(kc + 1) * 128],
                                wv_sb[:, k, :],
                                start=(k == 0), stop=(k == KC - 1),
                            )
                        nc.vector.tensor_copy(v_all[:, kc, :], ps[:, 0:HPC * DK])

            # ---- attention ----
            with tc.tile_pool(name="etp", bufs=2) as etp:
                for p in range(2):
                    qs = [q_pair[p][hh * 64:(hh + 1) * 64, :] for hh in range(2)]
                    ks = [k_pair[p][hh * 64:(hh + 1) * 64, :] for hh in range(2)]

                    with tc.tile_pool(name="sps", bufs=2, space="PSUM") as sps:
                        # normal layout: scores -> softmax -> attn rows out.
                        # One [128, 2048] psum per (head, row-chunk); its 1-4
                        # matmuls share lhsT (q chunk) -> single LDWEIGHTS.
                        for t in range(TC):
                            L = (t + 1) * 128
                            for hh in range(2):
                                h = 2 * p + hh
                                ps = sps.tile([128, 2048], F32, tag="sc",
                                              name=f"sc{hh}")
                                for n0 in range(0, L, 512):
                                    nn = min(512, L - n0)
                                    nc.tensor.matmul(
                                        ps[:, n0:n0 + nn],
                                        qs[hh][:, t * 128:(t + 1) * 128],
                                        ks[hh][:, n0:n0 + nn],
                                        start=True, stop=True,
                                    )
                                nc.vector.tensor_add(
                                    ps[:, L - 128:L], ps[:, L - 128:L],
                                    maska_sb[:])
                                e_sb = esbp.tile([128, S], F32, tag="e",
                                                 name=f"e{hh}")
                                ssum = small.tile([128, 1], F32, tag="st",
                                                  name=f"st{hh}")
                                nc.scalar.activation(
                                    out=e_sb[:, 0:L], in_=ps[:, 0:L],
                                    func=Exp, scale=SCALE,
                                    accum_out=ssum[:])
                                nc.vector.reciprocal(
                                    r_head[h][:, t:t + 1], ssum[:])
                                nc.vector.tensor_scalar_mul(
                                    e_sb[:, 0:L], e_sb[:, 0:L],
                                    r_head[h][:, t:t + 1])
                                nc.sync.dma_start(
                                    out=attn_o[h, t * 128:(t + 1) * 128, 0:L],
                                    in_=e_sb[:, 0:L])

                        # transposed layout: exp(scores.T) for the AV matmul
                        et_h = [etp.tile([128, ET_LEN], BF16, tag="et",
                                         name=f"et{hh}") for hh in range(2)]
                        for ci in range(TC):
                            Lc = (TC - ci) * 128
                            for hh in range(2):
                                ps = sps.tile([128, 2048], F32, tag="sc",
                                              name=f"tsc{hh}")
                                for n0 in range(0, Lc, 512):
                                    nn = min(512, Lc - n0)
                                    nc.tensor.matmul(
                                        ps[:, n0:n0 + nn],
                                        ks[hh][:, ci * 128:(ci + 1) * 128],
                                        qs[hh][:, ci * 128 + n0:
                                                ci * 128 + n0 + nn],
                                        start=True, stop=True,
                                    )
                                nc.vector.tensor_add(
                                    ps[:, 0:128], ps[:, 0:128], maskt_sb[:])
                                nc.scalar.activation(
                                    out=et_h[hh][:, ET_OFF[ci]:
                                                 ET_OFF[ci] + Lc],
                                    in_=ps[:, 0:Lc], func=Exp, scale=SCALE)

                    with tc.tile_pool(name="avps", bufs=2, space="PSUM") as avps:
                        # r broadcast [128, S]: rows 0:64 <- r of head 2p per
                        # token, rows 64:128 <- r of head 2p+1
                        rbc_sb = rbcp.tile([128, S], F32, tag="rbc")
                        for tg in range(4):
                            rps = avps.tile([128, 512], F32, tag="rp")
                            for tt in range(4):
                                t = tg * 4 + tt
                                for hh in range(2):
                                    tmp = small.tile([128, 128], F32, tag="tmp")
                                    nc.vector.tensor_scalar_mul(
                                        tmp[:], ident_sb[:],
                                        r_head[2 * p + hh][:, t:t + 1])
                                    nc.tensor.matmul(
                                        rps[hh * 64:(hh + 1) * 64,
                                            tt * 128:(tt + 1) * 128],
                                        ones_sb[:, 0:64], tmp[:],
                                        start=True, stop=True)
                            nc.vector.tensor_copy(
                                rbc_sb[:, tg * 512:(tg + 1) * 512], rps[:])

                        # attn @ V (transposed, unnormalized) + normalize
                        for n in range(4):
                            pt = avps.tile([128, 512], F32, tag="pt")
                            last_ci = 4 * n + 3
                            for ci in range(last_ci + 1):
                                col0 = max(0, ci * 128 - n * 512)
                                g0 = n * 512 + col0
                                ln = 512 - col0
                                for hh in range(2):
                                    nc.tensor.matmul(
                                        pt[hh * 64:(hh + 1) * 64, col0:512],
                                        v_all[:, ci, (2 * p + hh) * DK:
                                              (2 * p + hh + 1) * DK],
                                        et_h[hh][:, ET_OFF[ci] + g0 - ci * 128:
                                                 ET_OFF[ci] + g0 - ci * 128 + ln],
                                        start=(ci == 0), stop=(ci == last_ci),
                                        skip_group_check=True,
                                    )
                            nc.vector.tensor_tensor(
                                ao_pair[p][:, n * 512:(n + 1) * 512],
                                pt[:], rbc_sb[:, n * 512:(n + 1) * 512], Mult)

            # ---- output projection (partial, host sums across cores) ----
            with tc.tile_pool(name="wps", bufs=2, space="PSUM") as wps_pool:
                for t in range(TC):
                    osb = osbp.tile([128, D], F32, tag="osb")
                    wpss = [wps_pool.tile([128, 512], F32, tag=f"w{nh}",
                                          name=f"w{nh}") for nh in range(2)]
                    for p in range(2):
                        for nh in range(2):
                            nc.tensor.matmul(
                                wpss[nh][:],
                                ao_pair[p][:, t * 128:(t + 1) * 128],
                                wo_sb[:, p, nh * 512:(nh + 1) * 512],
                                start=(p == 0), stop=(p == 1),
                            )
                    for nh in range(2):
                        nc.vector.tensor_copy(
                            osb[:, nh * 512:(nh + 1) * 512], wpss[nh][:])
                    nc.sync.dma_start(
                        out=out_o[t * 128:(t + 1) * 128, :], in_=osb[:])

    nc.finalize()
    return nc


_NC = None


def _get_nc():
    global _NC
    if _NC is None:
        _NC = _build_nc()
    return _NC


def kernel(Q, K, V, W_q, W_k, W_v, W_o, b_o):
    Q = np.asarray(Q, dtype=np.float32)
    K = np.asarray(K, dtype=np.float32)
    V = np.asarray(V, dtype=np.float32)
    W_q = np.asarray(W_q, dtype=np.float32)
    W_k = np.asarray(W_k, dtype=np.float32)
    W_v = np.asarray(W_v, dtype=np.float32)
    W_o = np.asarray(W_o, dtype=np.float32)
    b_o = np.asarray(b_o, dtype=np.float32)

    bf = ml_dtypes.bfloat16
    ii = np.arange(128)
    maska = np.where(ii[None, :] <= ii[:, None], 0.0, NEG).astype(np.float32)
    maskt = maska.T.copy()
    ident = np.eye(128, dtype=np.float32)
    onesl = np.ones((128, 64), dtype=np.float32)

    xts = [[x[b].T.astype(bf) for x in (Q, K, V)] for b in range(B)]
    in_maps = []
    for c in range(N_CORES):
        b = c // 4
        g = c % 4
        osl = slice(g * HPC * DK, (g + 1) * HPC * DK)
        in_maps.append({
            "qt": xts[b][0], "kt": xts[b][1], "vt": xts[b][2],
            "wq": W_q[osl, :].T.astype(bf),
            "wk": W_k[osl, :].T.astype(bf),
            "wv": W_v[osl, :].T.astype(bf),
            "wo": W_o[:, osl].T.astype(bf),
            "maska": maska, "maskt": maskt, "ident": ident, "onesl": onesl,
        })

    nc = _get_nc()
    res = run_bass_kernel_spmd(nc, in_maps, core_ids=list(range(N_CORES)))

    attn = np.zeros((B, H, S, S), dtype=np.float32)
    out = np.zeros((B, S, D), dtype=np.float32)
    for c in range(N_CORES):
        b = c // 4
        g = c % 4
        attn[b, g * HPC:(g + 1) * HPC] = res.results[c]["attn"]
        out[b] += res.results[c]["outp"]
    # enforce exact zeros above the causal diagonal (the device never
    # writes there; this guards against non-zeroed output buffers)
    for t in range(TC):
        attn[:, :, t * 128:(t + 1) * 128, (t + 1) * 128:] = 0.0
    out += b_o
    return out, attn


# revision 14
# speedup vs baseline: 3.9842x; 1.0155x over previous
"""Multi-head causal attention (B=2, S=2048, D=1024, H=16) on 8 TRN2 cores.

Sharding: core c handles batch c//4 and heads 4*(c%4) .. 4*(c%4)+4
(data-parallel over batch x head-group). Inside a core:
  - qT/kT = W{q,k}_slice @ X.T   (pair-packed, [128=2*dk, S] tiles)
  - v     = X @ Wv_slice.T       ([S, 4*dk], key-chunk major)
  - per head: causal scores -> exp(scale*s) on ScalarE with fused row-sum
    accumulate -> normalize -> DMA the valid (lower-triangular) blocks of
    the attention matrix; scores are recomputed transposed for the
    attn @ V matmul (keys on partitions), normalized via a PE-broadcast
    of the row reciprocals, and fed into the W_o projection.
Host gathers the 8 attention shards and sums the 4 partial output
projections per batch.

Matmuls that share a stationary operand are emitted consecutively so
walrus folds their LDWEIGHTS and the PE streams them back-to-back
(ldw-opt is disabled in this toolchain, so every weight change costs a
serializing reload).
"""

import math

import numpy as np
import ml_dtypes

import concourse.bass as bass
import concourse.bass_utils as _bass_utils
from concourse import bacc
import concourse.mybir as mybir
import concourse.tile as tile
from concourse.bass_utils import run_bass_kernel_spmd

def _dedup_ldweights(nc):
    """Drop redundant PE weight reloads.

    The Tile lowering emits one InstLdweights per matmul even when the
    stationary operand is unchanged; each reload serializes the PE array
    (no fill/drain overlap). Consecutive sync-free Ldweights with an
    identical weights access pattern are no-ops -- remove them so
    same-weight matmul runs stream back-to-back.
    """
    removed = 0
    for f in nc.m.functions:
        for bb in f.blocks:
            keep = []
            last = None
            for ins in bb.instructions:
                if getattr(ins, "engine", None) == mybir.EngineType.PE:
                    nm = type(ins).__name__
                    if nm == "InstLdweights":
                        key = (str(ins.ins[0]), str(ins.tile_position),
                               str(ins.perf_mode), str(ins.is_transpose))
                        si = ins.sync_info
                        has_sync = bool(si) and (
                            getattr(si, "on_wait", None)
                            or getattr(si, "on_update", None))
                        if key == last and not has_sync:
                            removed += 1
                            continue
                        last = key
                    elif nm != "InstMatmult":
                        last = None
                keep.append(ins)
            if len(keep) != len(bb.instructions):
                bb.instructions[:] = keep
    return removed

F32 = mybir.dt.float32
BF16 = mybir.dt.bfloat16
Exp = mybir.ActivationFunctionType.Exp
Mult = mybir.AluOpType.mult

B, S, D, H, DK = 2, 2048, 1024, 16, 64
HPC = 4  # heads per core
N_CORES = 8
SCALE = 1.0 / math.sqrt(DK)
TC = S // 128  # 16 token chunks
NEG = -1.0e9

# packed offsets for the transposed-exp buffer: block ci covers tokens
# ci*128 .. S with (TC-ci)*128 columns
ET_OFF = [0] * (TC + 1)
for _ci in range(TC):
    ET_OFF[_ci + 1] = ET_OFF[_ci] + (TC - _ci) * 128
ET_LEN = ET_OFF[TC]


def _build_nc():
    nc = bacc.Bacc(None)

    qt = nc.declare_dram_parameter("qt", [D, S], BF16, isOutput=False)
    kt = nc.declare_dram_parameter("kt", [D, S], BF16, isOutput=False)
    vt = nc.declare_dram_parameter("vt", [D, S], BF16, isOutput=False)
    wq = nc.declare_dram_parameter("wq", [D, HPC * DK], BF16, isOutput=False)
    wk = nc.declare_dram_parameter("wk", [D, HPC * DK], BF16, isOutput=False)
    wv = nc.declare_dram_parameter("wv", [D, HPC * DK], BF16, isOutput=False)
    wo = nc.declare_dram_parameter("wo", [HPC * DK, D], BF16, isOutput=False)
    maska = nc.declare_dram_parameter("maska", [128, 128], F32, isOutput=False)
    maskt = nc.declare_dram_parameter("maskt", [128, 128], F32, isOutput=False)
    ident = nc.declare_dram_parameter("ident", [128, 128], F32, isOutput=False)
    onesl = nc.declare_dram_parameter("onesl", [128, 64], F32, isOutput=False)

    attn_o = nc.declare_dram_parameter("attn", [HPC, S, S], F32, isOutput=True)
    out_o = nc.declare_dram_parameter("outp", [S, D], F32, isOutput=True)

    qt_r = qt.rearrange("(o p) f -> p o f", p=128)
    kt_r = kt.rearrange("(o p) f -> p o f", p=128)
    vt_r = vt.rearrange("(o p) f -> p o f", p=128)
    wq_r = wq.rearrange("(o p) f -> p o f", p=128)
    wk_r = wk.rearrange("(o p) f -> p o f", p=128)
    wv_r = wv.rearrange("(o p) f -> p o f", p=128)
    wo_r = wo.rearrange("(o p) f -> p o f", p=128)
    KC = D // 128  # 8 contraction chunks for the projections

    with tile.TileContext(nc) as tc:
        with (
            tc.tile_pool(name="consts", bufs=1) as consts,
            tc.tile_pool(name="qkpool", bufs=1) as qkpool,
            tc.tile_pool(name="esb", bufs=4) as esbp,
            tc.tile_pool(name="small", bufs=6) as small,
            tc.tile_pool(name="rbcp", bufs=2) as rbcp,
            tc.tile_pool(name="aop", bufs=1) as aop,
            tc.tile_pool(name="osbp", bufs=2) as osbp,
            tc.tile_pool(name="psum", bufs=2, space="PSUM") as psum,
        ):
            # ---- constants ----
            wq_sb = consts.tile([128, KC, HPC * DK], BF16, tag="wq")
            wk_sb = consts.tile([128, KC, HPC * DK], BF16, tag="wk")
            wv_sb = consts.tile([128, KC, HPC * DK], BF16, tag="wv")
            wo_sb = consts.tile([128, 2, D], BF16, tag="wo")
            maska_sb = consts.tile([128, 128], F32, tag="ma")
            maskt_sb = consts.tile([128, 128], F32, tag="mt")
            ident_sb = consts.tile([128, 128], F32, tag="id")
            ones_sb = consts.tile([128, 64], F32, tag="on")
            nc.sync.dma_start(out=wq_sb[:], in_=wq_r[:])
            nc.sync.dma_start(out=wk_sb[:], in_=wk_r[:])
            nc.sync.dma_start(out=wv_sb[:], in_=wv_r[:])
            nc.sync.dma_start(out=wo_sb[:], in_=wo_r[:])
            nc.sync.dma_start(out=maska_sb[:], in_=maska[:])
            nc.sync.dma_start(out=maskt_sb[:], in_=maskt[:])
            nc.sync.dma_start(out=ident_sb[:], in_=ident[:])
            nc.sync.dma_start(out=ones_sb[:], in_=onesl[:])

            # persistent per-core activation tensors
            q_pair = [qkpool.tile([128, S], BF16, tag=f"qp{p}", name=f"qp{p}")
                      for p in range(2)]
            k_pair = [qkpool.tile([128, S], BF16, tag=f"kp{p}", name=f"kp{p}")
                      for p in range(2)]
            v_all = qkpool.tile([128, TC, HPC * DK], BF16, tag="va")
            r_head = [qkpool.tile([128, TC], F32, tag=f"r{h}", name=f"r{h}")
                      for h in range(HPC)]
            ao_pair = [aop.tile([128, S], BF16, tag=f"ao{p}", name=f"ao{p}")
                      for p in range(2)]

            # ---- projections ----
            with tc.tile_pool(name="proj_in", bufs=2) as proj_in:
                def load_xt(src_r):
                    t_sb = proj_in.tile([128, KC, S], BF16, tag="pin")
                    for o in range(KC):
                        nc.sync.dma_start(out=t_sb[:, o, :], in_=src_r[:, o, :])
                    return t_sb

                def proj_qk(x_sb, w_sb, dst_pair):
                    # one [128, 2048] accumulator per head pair; each
                    # K-chunk issues 4 same-weights matmuls
                    for p in range(2):
                        ps = psum.tile([128, 2048], F32, tag="sc", name="pp")
                        for k in range(KC):
                            lhs = w_sb[:, k, p * 128:(p + 1) * 128]
                            for nh in range(4):
                                nc.tensor.matmul(
                                    ps[:, nh * 512:(nh + 1) * 512],
                                    lhs,
                                    x_sb[:, k, nh * 512:(nh + 1) * 512],
                                    start=(k == 0), stop=(k == KC - 1),
                                )
                        nc.scalar.copy(dst_pair[p][:], ps[:])

                x_sb = load_xt(qt_r)
                proj_qk(x_sb, wq_sb, q_pair)
                x_sb = load_xt(kt_r)
                proj_qk(x_sb, wk_sb, k_pair)
                x_sb = load_xt(vt_r)
                for kc in range(TC):
                    ps = psum.tile([128, 2048], F32, tag="sc", name="vp")
                    for k in range(KC):
                        nc.tensor.matmul(
                            ps[:, 0:HPC * DK],
                            x_sb[:, k, kc * 128:(kc + 1) * 128],
                            wv_sb[:, k, :],
                            start=(k == 0), stop=(k == KC - 1),
                        )
                    nc.scalar.copy(v_all[:, kc, :], ps[:, 0:HPC * DK])

            # ---- attention ----
            with tc.tile_pool(name="etp", bufs=2) as etp:
                for p in range(2):
                    qs = [q_pair[p][hh * 64:(hh + 1) * 64, :] for hh in range(2)]
                    ks = [k_pair[p][hh * 64:(hh + 1) * 64, :] for hh in range(2)]

                    if True:
                        # normal layout: scores -> softmax -> attn rows out.
                        # The two heads' K=64 matmuls alternate PE row groups
                        # 0/64 chunk-by-chunk so they pack into disjoint
                        # subarrays; the causal mask is accumulated by an
                        # fp32 matmul (maskT.T @ I = maska) instead of a DVE
                        # add.
                        for t in range(TC):
                            L = (t + 1) * 128
                            pss = [psum.tile([128, 2048], F32, tag="sc",
                                             name=f"sc{hh}") for hh in range(2)]
                            for n0 in range(0, L, 512):
                                nn = min(512, L - n0)
                                for hh in range(2):
                                    nc.tensor.matmul(
                                        pss[hh][:, n0:n0 + nn],
                                        qs[hh][:, t * 128:(t + 1) * 128],
                                        ks[hh][:, n0:n0 + nn],
                                        start=True, stop=True,
                                    )
                            for hh in range(2):
                                nc.vector.tensor_add(
                                    pss[hh][:, L - 128:L],
                                    pss[hh][:, L - 128:L], maska_sb[:])
                            for hh in range(2):
                                h = 2 * p + hh
                                ps = pss[hh]
                                e_sb = esbp.tile([128, S], F32, tag="e",
                                                 name=f"e{hh}")
                                ssum = small.tile([128, 1], F32, tag="st",
                                                  name=f"st{hh}")
                                nc.scalar.activation(
                                    out=e_sb[:, 0:L], in_=ps[:, 0:L],
                                    func=Exp, scale=SCALE,
                                    accum_out=ssum[:])
                                nc.vector.reciprocal(
                                    r_head[h][:, t:t + 1], ssum[:])
                                nc.vector.tensor_scalar_mul(
                                    e_sb[:, 0:L], e_sb[:, 0:L],
                                    r_head[h][:, t:t + 1])
                                nc.sync.dma_start(
                                    out=attn_o[h, t * 128:(t + 1) * 128, 0:L],
                                    in_=e_sb[:, 0:L])

                        # transposed layout: exp(scores.T) for the AV matmul
                        et_h = [etp.tile([128, ET_LEN], BF16, tag="et",
                                         name=f"et{hh}") for hh in range(2)]
                        for ci in range(TC):
                            Lc = (TC - ci) * 128
                            pss = [psum.tile([128, 2048], F32, tag="sc",
                                             name=f"tsc{hh}") for hh in range(2)]
                            for n0 in range(0, Lc, 512):
                                nn = min(512, Lc - n0)
                                for hh in range(2):
                                    nc.tensor.matmul(
                                        pss[hh][:, n0:n0 + nn],
                                        ks[hh][:, ci * 128:(ci + 1) * 128],
                                        qs[hh][:, ci * 128 + n0:
                                                ci * 128 + n0 + nn],
                                        start=True, stop=True,
                                    )
                            for hh in range(2):
                                nc.vector.tensor_add(
                                    pss[hh][:, 0:128], pss[hh][:, 0:128],
                                    maskt_sb[:])
                            for hh in range(2):
                                nc.scalar.activation(
                                    out=et_h[hh][:, ET_OFF[ci]:
                                                 ET_OFF[ci] + Lc],
                                    in_=pss[hh][:, 0:Lc], func=Exp,
                                    scale=SCALE)

                    if True:
                        # r broadcast [128, S]: rows 0:64 <- r of head 2p per
                        # token, rows 64:128 <- r of head 2p+1
                        rbc_sb = rbcp.tile([128, S], F32, tag="rbc")
                        for tg in range(4):
                            rps = psum.tile([128, 2048], F32, tag="sc", name="rp")[:, 0:512]
                            for tt in range(4):
                                t = tg * 4 + tt
                                for hh in range(2):
                                    tmp = small.tile([128, 128], F32, tag="tmp")
                                    nc.vector.tensor_scalar_mul(
                                        tmp[:], ident_sb[:],
                                        r_head[2 * p + hh][:, t:t + 1])
                                    nc.tensor.matmul(
                                        rps[hh * 64:(hh + 1) * 64,
                                            tt * 128:(tt + 1) * 128],
                                        ones_sb[:, 0:64], tmp[:],
                                        start=True, stop=True)
                            nc.vector.tensor_copy(
                                rbc_sb[:, tg * 512:(tg + 1) * 512], rps[:])

                        # attn @ V (transposed, unnormalized) + normalize
                        for n in range(4):
                            pt = psum.tile([128, 2048], F32, tag="sc", name="pt")[:, 0:512]
                            last_ci = 4 * n + 3
                            for ci in range(last_ci + 1):
                                col0 = max(0, ci * 128 - n * 512)
                                g0 = n * 512 + col0
                                ln = 512 - col0
                                for hh in range(2):
                                    nc.tensor.matmul(
                                        pt[hh * 64:(hh + 1) * 64, col0:512],
                                        v_all[:, ci, (2 * p + hh) * DK:
                                              (2 * p + hh + 1) * DK],
                                        et_h[hh][:, ET_OFF[ci] + g0 - ci * 128:
                                                 ET_OFF[ci] + g0 - ci * 128 + ln],
                                        start=(ci == 0), stop=(ci == last_ci),
                                        skip_group_check=True,
                                    )
                            nc.vector.tensor_tensor(
                                ao_pair[p][:, n * 512:(n + 1) * 512],
                                pt[:], rbc_sb[:, n * 512:(n + 1) * 512], Mult)

            # ---- output projection (partial, host sums across cores) ----
            for t in range(TC):
                osb = osbp.tile([128, D], F32, tag="osb")
                wps = psum.tile([128, 2048], F32, tag="sc", name="wp")[:, 0:1024]
                for p in range(2):
                    for nh in range(2):
                        nc.tensor.matmul(
                            wps[:, nh * 512:(nh + 1) * 512],
                            ao_pair[p][:, t * 128:(t + 1) * 128],
                            wo_sb[:, p, nh * 512:(nh + 1) * 512],
                            start=(p == 0), stop=(p == 1),
                        )
                nc.scalar.copy(osb[:], wps[:])
                nc.sync.dma_start(
                    out=out_o[t * 128:(t + 1) * 128, :], in_=osb[:])

    _dedup_ldweights(nc)
    nc.finalize()
    return nc


_NC = None


def _get_nc():
    global _NC
    if _NC is None:
        _NC = _build_nc()
    return _NC


def kernel(Q, K, V, W_q, W_k, W_v, W_o, b_o):
    Q = np.asarray(Q, dtype=np.float32)
    K = np.asarray(K, dtype=np.float32)
    V = np.asarray(V, dtype=np.float32)
    W_q = np.asarray(W_q, dtype=np.float32)
    W_k = np.asarray(W_k, dtype=np.float32)
    W_v = np.asarray(W_v, dtype=np.float32)
    W_o = np.asarray(W_o, dtype=np.float32)
    b_o = np.asarray(b_o, dtype=np.float32)

    bf = ml_dtypes.bfloat16
    ii = np.arange(128)
    maska = np.where(ii[None, :] <= ii[:, None], 0.0, NEG).astype(np.float32)
    maskt = maska.T.copy()
    ident = np.eye(128, dtype=np.float32)
    onesl = np.ones((128, 64), dtype=np.float32)

    xts = [[x[b].T.astype(bf) for x in (Q, K, V)] for b in range(B)]
    in_maps = []
    for c in range(N_CORES):
        b = c // 4
        g = c % 4
        osl = slice(g * HPC * DK, (g + 1) * HPC * DK)
        in_maps.append({
            "qt": xts[b][0], "kt": xts[b][1], "vt": xts[b][2],
            "wq": W_q[osl, :].T.astype(bf),
            "wk": W_k[osl, :].T.astype(bf),
            "wv": W_v[osl, :].T.astype(bf),
            "wo": W_o[:, osl].T.astype(bf),
            "maska": maska, "maskt": maskt, "ident": ident, "onesl": onesl,
        })

    nc = _get_nc()
    res = run_bass_kernel_spmd(nc, in_maps, core_ids=list(range(N_CORES)))

    attn = np.zeros((B, H, S, S), dtype=np.float32)
    out = np.zeros((B, S, D), dtype=np.float32)
    for c in range(N_CORES):
        b = c // 4
        g = c % 4
        attn[b, g * HPC:(g + 1) * HPC] = res.results[c]["attn"]
        out[b] += res.results[c]["outp"]
    # enforce exact zeros above the causal diagonal (the device never
    # writes there; this guards against non-zeroed output buffers)
    for t in range(TC):
        attn[:, :, t * 128:(t + 1) * 128, (t + 1) * 128:] = 0.0
    out += b_o
    return out, attn
